# revision 2
# baseline (speedup 1.0000x reference)
"""Self-contained TRN2 Bass kernel for causal multi-head attention.

Problem: x[4, 2048, 1024], Wq/Wk/Wv[1024, 1024], H=16 heads, causal softmax.
Sharding: 8 NeuronCores = 4 batches x 2 head-groups (8 heads each).
kernel(**inputs) takes full inputs, shards, runs SPMD on cores 0-7, gathers.

Per-core program (all matmuls fp32r — full PE rate at free dim >= 256):
  xT [1024, 2048] is fed pre-transposed from the host (input layout prep).
  QT_p/KT_p [128, 2048] per head-pair p (dh on partitions, heads 2p/2p+1 at
      partitions 0:64 / 64:128).
  V [16 s-tiles][128, 4*65]: natural layout, 4 heads per half, plus a ones
      column per head (softmax denominator accumulates through the same
      matmul as the context).
  scoresT[k, q]: per k-tile PAIR, 4 row-tiled matmuls (the two heads use
      PE array rows 0:64 / 64:128 concurrently) -> psum [128, 2048]
      -> one ACT exp (fused *= 0.125) -> es sbuf tile.
  diagonal blocks: DVE multiply by a [128,128] triangular 0/1 mask; fully
      masked columns are skipped by slicing the ctx matmul rhs instead.
  ctx'^T [65, 512] accumulates per (pair, q-block); row 64 = denominators.
  PE-transpose [65,128] blocks -> [128, 65]; DVE reciprocal + tensor_scalar
      normalizes straight out of PSUM into the output staging tile; DMA out.
Projections for pair p+1 are emitted as "fillers" inside pair p's attention
k-loop so PE proj work overlaps the ACT-bound exp stream.
"""

import sys

if "/opt/trn_rl_repo" not in sys.path:
    sys.path.insert(0, "/opt/trn_rl_repo")

from contextlib import ExitStack

import numpy as np

import concourse.bass as bass  # noqa: F401
import concourse.mybir as mybir
import concourse.tile as tile
from concourse import bacc
from concourse.bass_utils import run_bass_kernel_spmd
from concourse.masks import make_identity, make_upper_triangular

F32 = mybir.dt.float32
F32R = mybir.dt.float32r
EXP = mybir.ActivationFunctionType.Exp

B = 4
S = 2048  # sequence length
DM = 1024  # model dim
DH = 512  # per-core head-dim total (8 heads x 64)
HD = 64
NPAIR = 4  # head pairs per core
ST = S // 128
DC = DM // 128
NJ = S // 512
N_CORES = 8


def build_nc():
    nc = bacc.Bacc("TRN2", target_bir_lowering=False, debug=False)
    XT = nc.dram_tensor("xt", [DM, S], F32, kind="ExternalInput")
    WQ = nc.dram_tensor("wq", [DM, DH], F32, kind="ExternalInput")
    WK = nc.dram_tensor("wk", [DM, DH], F32, kind="ExternalInput")
    WV = nc.dram_tensor("wv", [DM, DH], F32, kind="ExternalInput")
    ONES = nc.dram_tensor("ones", [128, 8], F32, kind="ExternalInput")
    OUT = nc.dram_tensor("out", [S, DH], F32, kind="ExternalOutput")

    with ExitStack() as ctx:
        tc = ctx.enter_context(tile.TileContext(nc))

        const = ctx.enter_context(tc.tile_pool(name="const", bufs=1))
        ident = const.tile([128, 128], F32)
        make_identity(nc, ident[:])
        tri = const.tile([128, 128], F32)  # tri[k, q] = 1.0 if k <= q else 0
        make_upper_triangular(nc, tri[:], val=1.0, diag=True)

        xt_pool = ctx.enter_context(tc.tile_pool(name="xt", bufs=1))
        w_pool = ctx.enter_context(tc.tile_pool(name="w", bufs=1))
        qk_pool = ctx.enter_context(tc.tile_pool(name="qk", bufs=1))
        v_pool = ctx.enter_context(tc.tile_pool(name="v", bufs=1))
        es_pool = ctx.enter_context(tc.tile_pool(name="es", bufs=3))
        cst_pool = ctx.enter_context(tc.tile_pool(name="cst", bufs=4))
        out_pool = ctx.enter_context(tc.tile_pool(name="po", bufs=2))
        rc_pool = ctx.enter_context(tc.tile_pool(name="rc", bufs=2))
        psES = ctx.enter_context(tc.tile_pool(name="psES", bufs=1, space="PSUM"))
        psCtx = ctx.enter_context(tc.tile_pool(name="psCtx", bufs=2, space="PSUM"))
        psAux = ctx.enter_context(tc.tile_pool(name="psAux", bufs=2, space="PSUM"))

        # xT: direct DMA of the pre-transposed x, chunked so the first
        # projection chains can start after ~2MB.
        xT = [xt_pool.tile([128, S], F32R, name=f"xT{c}") for c in range(DC)]
        for sc in range(4):
            for c in range(DC):
                nc.sync.dma_start(
                    xT[c][:, sc * 512 : (sc + 1) * 512],
                    XT[c * 128 : (c + 1) * 128, sc * 512 : (sc + 1) * 512].bitcast(
                        F32R
                    ),
                )

        wq_t, wk_t, wv_t = {}, {}, {}

        def load_wqk(p):
            wq_t[p] = w_pool.tile([128, DC, 128], F32R, name="wq", tag="wq", bufs=2)
            wk_t[p] = w_pool.tile([128, DC, 128], F32R, name="wk", tag="wk", bufs=2)
            nc.sync.dma_start(
                wq_t[p][:],
                WQ[:, p * 128 : (p + 1) * 128]
                .bitcast(F32R)
                .rearrange("(c p) n -> p c n", p=128),
            )
            nc.sync.dma_start(
                wk_t[p][:],
                WK[:, p * 128 : (p + 1) * 128]
                .bitcast(F32R)
                .rearrange("(c p) n -> p c n", p=128),
            )

        def load_wv(half):
            wv_t[half] = w_pool.tile([128, DC, 256], F32R, name="wv", tag="wv", bufs=1)
            nc.sync.dma_start(
                wv_t[half][:],
                WV[:, half * 256 : (half + 1) * 256]
                .bitcast(F32R)
                .rearrange("(c p) n -> p c n", p=128),
            )

        load_wqk(0)
        load_wv(0)

        QT, KT, VT = {}, {}, {}
        copy_eng = [0]

        fillers = []

        def proj_chain(p, which, sc):
            wt = (wq_t if which == "q" else wk_t)[p]
            dstmap = QT if which == "q" else KT
            if sc == 0:
                dstmap[p] = qk_pool.tile(
                    [128, S], F32R, name=f"{which}T", tag=f"{which}T", bufs=2
                )
            acc = psAux.tile([128, 512], F32, name="prjp", tag="aux")
            for c in range(DC):
                nc.tensor.matmul(
                    acc[:],
                    wt[:, c, :],
                    xT[c][:, sc * 512 : (sc + 1) * 512],
                    start=(c == 0),
                    stop=(c == DC - 1),
                )
            dst = dstmap[p][:, sc * 512 : (sc + 1) * 512]
            if copy_eng[0] == 0:
                nc.scalar.copy(dst, acc[:])
            else:
                nc.vector.tensor_copy(dst, acc[:])
            copy_eng[0] ^= 1

        def v_chain(half, st):
            vt = v_pool.tile([128, 4, 65], F32R, name=f"vt{st}", tag=f"vt{st}", bufs=2)
            VT[(half, st)] = vt
            acc = psAux.tile([128, 256], F32, name="prjv", tag="aux")
            for c in range(DC):
                nc.tensor.matmul(
                    acc[:],
                    xT[c][:, st * 128 : (st + 1) * 128],
                    wv_t[half][:, c, :],
                    start=(c == 0),
                    stop=(c == DC - 1),
                )
            nc.vector.tensor_copy(
                vt[:, :, 0:64], acc[:].rearrange("p (h c) -> p h c", h=4)
            )
            nc.sync.dma_start(
                vt[:, :, 64:65],
                ONES[:, 0:4].bitcast(F32R).rearrange("p (h o) -> p h o", o=1),
            )

        def emit_fillers(n):
            for _ in range(n):
                if fillers:
                    fillers.pop(0)()

        for sc in range(4):
            proj_chain(0, "q", sc)
            proj_chain(0, "k", sc)
        for st in range(ST):
            v_chain(0, st)

        for p in range(NPAIR):
            if p + 1 < NPAIR:
                load_wqk(p + 1)
                for sc in range(4):
                    fillers.append(lambda p1=p + 1, sc=sc: proj_chain(p1, "q", sc))
                    fillers.append(lambda p1=p + 1, sc=sc: proj_chain(p1, "k", sc))
            if p == 1:
                load_wv(1)
                for st in range(ST):
                    fillers.append(lambda st=st: v_chain(1, st))

            half = p // 2
            qt, kt = QT[p], KT[p]
            ngrp_total = sum(2 * J + 2 for J in range(NJ))
            n_fill = len(fillers)
            filled = 0
            seen = 0

            for J in range(NJ):
                ctx0 = psCtx.tile([65, 512], F32, name="ctx0", tag="ctx")
                ctx1 = psCtx.tile([65, 512], F32, name="ctx1", tag="ctx")
                nkt = 4 * J + 4
                qsl = slice(J * 512, (J + 1) * 512)
                for g in range(nkt // 2):
                    es_ps = psES.tile([128, 2048], F32, name="esp", tag="esp")
                    for u in range(2):
                        t = 2 * g + u
                        ksl = slice(t * 128, (t + 1) * 128)
                        nc.tensor.matmul(
                            es_ps[:, u * 1024 : u * 1024 + 512],
                            kt[0:64, ksl],
                            qt[0:64, qsl],
                            start=True,
                            stop=True,
                        )
                        nc.tensor.matmul(
                            es_ps[:, u * 1024 + 512 : u * 1024 + 1024],
                            kt[64:128, ksl],
                            qt[64:128, qsl],
                            start=True,
                            stop=True,
                        )
                    es = es_pool.tile([128, 2048], F32R, name="es", tag="es")
                    nc.scalar.activation(es[:], es_ps[:], EXP, bias=0.0, scale=0.125)
                    for u in range(2):
                        t = 2 * g + u
                        tloc = t - 4 * J
                        if tloc >= 0:  # diagonal k-tile: mask boundary square
                            for h in range(2):
                                sq = slice(
                                    u * 1024 + h * 512 + tloc * 128,
                                    u * 1024 + h * 512 + (tloc + 1) * 128,
                                )
                                nc.vector.tensor_mul(
                                    es[:, sq], es[:, sq], tri[:].bitcast(F32R)
                                )
                            off = tloc * 128
                        else:
                            off = 0
                        vt = VT[(half, t)]
                        hh0, hh1 = (p % 2) * 2, (p % 2) * 2 + 1
                        nc.tensor.matmul(
                            ctx0[:, off:512],
                            vt[:, hh0, :],
                            es[:, u * 1024 + off : u * 1024 + 512],
                            start=(t == 0),
                            stop=(t == nkt - 1),
                        )
                        nc.tensor.matmul(
                            ctx1[:, off:512],
                            vt[:, hh1, :],
                            es[:, u * 1024 + 512 + off : u * 1024 + 1024],
                            start=(t == 0),
                            stop=(t == nkt - 1),
                        )
                    seen += 1
                    want = (seen * n_fill) // ngrp_total
                    emit_fillers(want - filled)
                    filled = max(filled, want)

                ps_out = out_pool.tile([128, 4, 128], F32, name="po", tag="po")
                for h, cpsum in enumerate((ctx0, ctx1)):
                    cst = cst_pool.tile([65, 512], F32, name="cst", tag="cst")
                    nc.vector.tensor_copy(cst[:], cpsum[:])
                    tp = psAux.tile([128, 260], F32, name="ctp", tag="aux")
                    for tau in range(4):
                        nc.tensor.transpose(
                            tp[:, tau * 65 : (tau + 1) * 65],
                            cst[:, tau * 128 : (tau + 1) * 128],
                            ident[0:65, 0:65],
                        )
                    rc = rc_pool.tile([128, 4], F32, name="rc", tag="rc")
                    tp4 = tp[:].rearrange("p (t c) -> p t c", t=4)
                    nc.vector.reciprocal(rc[:], tp4[:, :, 64])
                    for tau in range(4):
                        nc.vector.tensor_scalar_mul(
                            ps_out[:, tau, h * 64 : (h + 1) * 64],
                            tp4[:, tau, 0:64],
                            rc[:, tau : tau + 1],
                        )
                nc.sync.dma_start(
                    OUT[J * 512 : (J + 1) * 512, p * 128 : (p + 1) * 128].rearrange(
                        "(t r) c -> r t c", t=4
                    ),
                    ps_out[:],
                )
            emit_fillers(len(fillers))

    nc.compile()
    return nc


_NC_CACHE = None


def _get_nc():
    global _NC_CACHE
    if _NC_CACHE is None:
        _NC_CACHE = build_nc()
    return _NC_CACHE


def make_in_maps(x, Wq, Wk, Wv):
    ones = np.ones((128, 8), np.float32)
    in_maps = []
    for c in range(N_CORES):
        b, g = c // 2, c % 2
        cols = slice(g * DH, (g + 1) * DH)
        in_maps.append(
            {
                "xt": np.ascontiguousarray(x[b].T),
                "wq": np.ascontiguousarray(Wq[:, cols]),
                "wk": np.ascontiguousarray(Wk[:, cols]),
                "wv": np.ascontiguousarray(Wv[:, cols]),
                "ones": ones,
            }
        )
    return in_maps


def run_sharded(x, Wq, Wk, Wv, trace=False):
    nc = _get_nc()
    in_maps = make_in_maps(x, Wq, Wk, Wv)
    res = run_bass_kernel_spmd(nc, in_maps, list(range(N_CORES)), trace=trace)
    out = np.zeros((B, S, DM), dtype=np.float32)
    for c in range(N_CORES):
        b, g = c // 2, c % 2
        out[b, :, g * DH : (g + 1) * DH] = res.results[c]["out"]
    return out, res.exec_time_ns


def kernel(x, Wq, Wk, Wv):
    x = np.asarray(x, dtype=np.float32)
    Wq = np.asarray(Wq, dtype=np.float32)
    Wk = np.asarray(Wk, dtype=np.float32)
    Wv = np.asarray(Wv, dtype=np.float32)
    out, _ = run_sharded(x, Wq, Wk, Wv, trace=False)
    return out
